# revision 58
# baseline (speedup 1.0000x reference)
"""Trainium2 Bass kernel for 16-head causal MultiHeadAttention.

Problem shapes (hardcoded): x [4, 2048, 1024], Wq/Wk/Wv [1024, 1024],
Wc [1024, 1024], bc [1024].  Output [4, 2048, 1024].

Sharding: 8 cores = (batch b in 0..3) x (head-group g in 0..1).
Each core computes 8 heads (512 of the 1024 hidden dims) for one batch
element, including its partial c_proj contribution.  The host sums the
two partials per batch (fp32) and adds the bias.

All matmul operands are bf16 (host-cast inputs; fp32 PSUM accumulate;
bf16 output summed in fp32 on the host) — rel err ~6e-3 vs the 2e-2
gate, and bf16 halves LDWEIGHTS/SBUF/DMA traffic and power-throttling
pressure vs fp32r.

Per-core kernel:
  P1:  one pass over x^T computing Q^T, K^T = W^T @ x_b^T [512, 2048]
       and V = x_b @ Wv_g (stored with a ones column per head).
       Startup: 2-kc-wide x/weight tiles, weights spread over the
       scalar/gpsimd/sync DMA rings interleaved with x so the first
       matmul only waits for its own slices.
  P2:  per (head-pair, q-chunk, k-chunk): the two heads' 64-row S^T
       matmuls are adjacent and PE-array-tile paired (rows 0-63/64-127
       concurrently) -> joint exp over both heads (scale 1/8) -> causal
       triangle mask (diagonal blocks only) -> O^T/denominator
       accumulate via [V | 1] stationary.  Diagonal k-chunks truncate
       the fully-masked leading q-columns, shrinking S/exp/O by ~15%.
       Eviction reads PSUM once (fast bank recycling), then reciprocal
       on the 1-row denominator + 1 KB DRAM-bounce broadcast of the
       bf16 reciprocal, all off the PSUM critical path.
  P3:  partial out = O @ Wc_g   (O^T chunks are the matmul stationaries)
"""

import numpy as np

B, T, C = 4, 2048, 1024
H_PER_CORE = 8       # heads per core
HL = 512             # local head width  (8 heads * 64)
D = 64               # head dim
QC = 512             # q-chunk width (moving free dim)
NQC = T // QC        # 4
NKC = T // 128       # 16
N_CORES = 8

_CACHE = {}


def _emit(nc, tc, tile, mybir, io):
    import concourse.bass as bass
    f32, bf16 = mybir.dt.float32, mybir.dt.bfloat16
    Exp = mybir.ActivationFunctionType.Exp
    xT, wq, wk, wv, wc, maskw, out = (
        io["xT"], io["wq"], io["wk"], io["wv"], io["wc"],
        io["maskw"], io["out"],
    )

    from contextlib import ExitStack

    with ExitStack() as ctx:
        persist = ctx.enter_context(tc.tile_pool(name="persist", bufs=1))
        # Q^T / K^T / O^T: [512 rows, 2048 toks] as [128, 4 chunks, 2048]
        qt = persist.tile([128, 4, T], bf16)
        kt = persist.tile([128, 4, T], bf16)
        # V': [2048 toks, 8 heads x (64 dims + ones col)] as [128, 16, 520]
        vp = persist.tile([128, NKC, H_PER_CORE * (D + 1)], bf16)
        vp4 = vp.rearrange("p mt (h c) -> p mt h c", c=D + 1)
        # ones column per head (denominator accumulator in the O matmuls)
        nc.gpsimd.memset(vp4[:, :, :, D], 1.0)

        # ------- Phase 1: Q^T, K^T, V in one pass over x^T -------
        with (
            tc.tile_pool(name="wqk", bufs=1) as wpool,
            tc.tile_pool(name="xtp", bufs=8) as xtp,
            tc.tile_pool(name="ps1", bufs=6, space="PSUM") as ps1,
        ):
            wq_t = [wpool.tile([128, 2, HL], bf16, tag=f"wq{i}", name=f"wq{i}")
                    for i in range(4)]
            wk_t = [wpool.tile([128, 2, HL], bf16, tag=f"wk{i}", name=f"wk{i}")
                    for i in range(4)]
            wv_t = [wpool.tile([128, 2, HL], bf16, tag=f"wv{i}", name=f"wv{i}")
                    for i in range(4)]

            def load_xt(n, with_weights=False):
                """x^T [1024, 512-tok chunk n] as 4 two-kc tiles on the sync
                DMA ring; on the first chunk the weight loads ride the
                scalar/vector/tensor rings in parallel, interleaved so
                matmul kc only waits for its own slice of x and W."""
                xts = []
                for i in range(4):
                    t = xtp.tile([128, 2, QC], bf16, tag="xt")
                    # xT is pre-chunked host-side to [NQC, C, QC] so this
                    # reads one fully contiguous 256 KB block
                    nc.sync.dma_start(
                        out=t,
                        in_=xT[n, i * 256:(i + 1) * 256, :]
                        .rearrange("(c p) m -> p c m", p=128))
                    if with_weights:
                        sl = slice(i * 256, (i + 1) * 256)
                        nc.scalar.dma_start(
                            out=wq_t[i],
                            in_=wq[sl, :].rearrange("(c p) m -> p c m", p=128))
                        nc.gpsimd.dma_start(
                            out=wk_t[i],
                            in_=wk[sl, :].rearrange("(c p) m -> p c m", p=128))
                    xts.append(t)
                if with_weights:
                    # wv rides the sync ring after x: needed last (V matmuls
                    # follow Q/K per n) — keeps all three rings ~1 MB each
                    for i in range(4):
                        sl = slice(i * 256, (i + 1) * 256)
                        nc.sync.dma_start(
                            out=wv_t[i],
                            in_=wv[sl, :].rearrange("(c p) m -> p c m", p=128))
                return lambda kc: xts[kc // 2][:, kc % 2, :]

            def wsl(wt, kc):
                return wt[kc // 2][:, kc % 2, :]

            for n in range(NQC):
                xs = load_xt(n, with_weights=(n == 0))
                # all Q tiles first: ~13us of PE work gated only on wq+x,
                # hiding the slower wk (SWDGE) / wv transfer latency
                for mc in range(4):
                    pq = ps1.tile([128, QC], f32, tag="p1")
                    for kc in range(8):
                        nc.tensor.matmul(
                            out=pq[:],
                            lhsT=wsl(wq_t, kc)[:, mc * 128:(mc + 1) * 128],
                            rhs=xs(kc), start=(kc == 0), stop=(kc == 7))
                    nc.scalar.copy(qt[:, mc, n * QC:(n + 1) * QC], pq[:])
                for mc in range(4):
                    pk = ps1.tile([128, QC], f32, tag="p1")
                    for kc in range(8):
                        nc.tensor.matmul(
                            out=pk[:],
                            lhsT=wsl(wk_t, kc)[:, mc * 128:(mc + 1) * 128],
                            rhs=xs(kc), start=(kc == 0), stop=(kc == 7))
                    nc.vector.tensor_copy(kt[:, mc, n * QC:(n + 1) * QC], pk[:])
                for mt in range(4):
                    gm = n * 4 + mt           # global token chunk (0..15)
                    pv = ps1.tile([128, HL], f32, tag="p1")
                    for kc in range(8):
                        nc.tensor.matmul(
                            out=pv[:], lhsT=xs(kc)[:, mt * 128:(mt + 1) * 128],
                            rhs=wsl(wv_t, kc), start=(kc == 0), stop=(kc == 7))
                    nc.vector.tensor_copy(
                        vp4[:, gm, :, 0:D],
                        pv.rearrange("p (h d) -> p h d", d=D))


        # ---------------- Phase 2: attention ----------------
        with tc.tile_pool(name="otp", bufs=1) as otpool, \
             tc.tile_pool(name="wcp", bufs=1) as wcpool:
            ot = otpool.tile([128, 4, T], bf16)
            # preload Wc during P2 (used in P3)
            wc_sb = wcpool.tile([128, 4, C], bf16)
            nc.sync.dma_start(
                out=wc_sb, in_=wc.rearrange("(kd p) m -> p kd m", p=128))

            with (
                tc.tile_pool(name="mk", bufs=1) as mkpool,
                tc.tile_pool(name="etp", bufs=8) as etp,
                tc.tile_pool(name="smp", bufs=6) as smp,
                tc.tile_pool(name="drp", bufs=8, space="DRAM") as drp,
                tc.tile_pool(name="psw", bufs=3, space="PSUM") as psw,
                tc.tile_pool(name="pso", bufs=2, space="PSUM") as pso,
            ):
                # causal triangle for the 128-wide diagonal block,
                # duplicated so one tensor_mul masks a head pair at once
                mask_sb = mkpool.tile([128, 2, 128], bf16)
                nc.sync.dma_start(out=mask_sb, in_=maskw)

                def emit_chunk(ha, hb, qc, po_a, po_b, kc, K):
                    """One k-chunk for a head pair: adjacent 64-row S-mms
                    (PE-tile paired), joint exp + mask, two O-mms.

                    Diagonal chunks (d >= 0) skip the fully-masked leading
                    q-columns: only q >= 128*d can attend to this chunk, so
                    the S/exp/O work all shrink; just the leading 128-wide
                    block needs the causal triangle mask."""
                    d = kc - 4 * qc
                    off = max(d, 0) * 128
                    N = QC - off
                    pw = psw.tile([128, 2, QC], f32, tag="pw")
                    for j, h in ((0, ha), (1, hb)):
                        r0 = (h % 2) * 64
                        chh = h // 2
                        # 64-row array tiling: even heads use PE rows 0-63,
                        # odd heads rows 64-127 — the two adjacent S-matmuls
                        # run concurrently on the two halves.
                        nc.tensor.matmul(
                            out=pw[:, j, 0:N],
                            lhsT=kt[r0:r0 + 64, chh, kc * 128:(kc + 1) * 128],
                            rhs=qt[r0:r0 + 64, chh,
                                   qc * QC + off:(qc + 1) * QC],
                            start=True, stop=True, tile_position=(r0, 0))
                    ew = etp.tile([128, 2, QC], bf16, tag="et")
                    nc.scalar.activation(ew[:, :, 0:N], pw[:, :, 0:N],
                                         Exp, scale=0.125)
                    if d >= 0:               # diagonal block: causal triangle
                        nc.vector.tensor_mul(
                            ew[:, :, 0:128], ew[:, :, 0:128], mask_sb[:])
                    for j, h, po in ((0, ha, po_a), (1, hb, po_b)):
                        nc.tensor.matmul(
                            out=po[0:D + 1, off:QC],
                            lhsT=vp[:, kc, h * (D + 1):(h + 1) * (D + 1)],
                            rhs=ew[:, j, 0:N],
                            start=(kc == 0), stop=(kc == K - 1))

                onesrow = mkpool.tile([1, 64], bf16)
                nc.vector.memset(onesrow[:], 1.0)

                def evict(h, qc, po, fast=False):
                    """PSUM eviction with a single po read (frees the PSUM
                    bank after one op) + off-critical-path normalization.

                    Reciprocal runs on the single denominator row, then the
                    bf16 reciprocal is broadcast via a tiny DRAM bounce
                    (1 KB) instead of broadcasting the raw denominator and
                    computing 128 reciprocal rows.  The last evicts gate the
                    final c_proj tiles, so `fast` broadcasts via a small PE
                    matmul (the PE is idle there) instead of two DMA hops."""
                    r0 = (h % 2) * 64
                    chh = h // 2
                    ot_slice = ot[r0:r0 + 64, chh, qc * QC:(qc + 1) * QC]
                    og = smp.tile([65, QC], f32, tag="og")
                    nc.vector.tensor_copy(og[:], po[0:D + 1, :])
                    d1 = smp.tile([1, QC], f32, tag="d1")
                    nc.vector.tensor_copy(d1[:], og[D:D + 1, :])
                    nc.vector.reciprocal_approx_fast(d1[:], d1[:])
                    dr = smp.tile([1, QC], bf16, tag="dr")
                    nc.vector.tensor_copy(dr[:], d1[:])
                    if fast:
                        pb = psw.tile([128, 2, QC], f32, tag="pw")
                        nc.tensor.matmul(
                            out=pb[0:64, 0, :], lhsT=onesrow[:],
                            rhs=dr[:], start=True, stop=True)
                        nc.vector.tensor_mul(ot_slice, og[0:64, :],
                                             pb[0:64, 0, :])
                        return
                    scr = drp.tile([1, QC], bf16, tag="scr")
                    nc.sync.dma_start(out=scr[:], in_=dr[:])
                    db = smp.tile([64, QC], bf16, tag="db")
                    s0 = scr[:]
                    nc.gpsimd.dma_start(
                        out=db[:],
                        in_=bass.AP(tensor=s0.tensor, offset=s0.offset,
                                    ap=[[0, 64], [1, QC]]))
                    nc.vector.tensor_mul(ot_slice, og[0:64, :], db[:])

                # Head pairs in chunk-lockstep so the two heads' 64-row
                # S-matmuls are adjacent and fill both PE array halves.
                for hp in range(H_PER_CORE // 2):
                    ha, hb = 2 * hp, 2 * hp + 1
                    for qc in range(NQC):
                        K = 4 * qc + 4      # causal k-chunks for this q-chunk
                        po_a = pso.tile([128, QC], f32, tag="po")
                        po_b = pso.tile([128, QC], f32, tag="po")
                        for kc in range(K):
                            emit_chunk(ha, hb, qc, po_a, po_b, kc, K)
                        evict(ha, qc, po_a)
                        evict(hb, qc, po_b)

            # ---------------- Phase 3: c_proj partial ----------------
            with (
                tc.tile_pool(name="stp", bufs=4) as stp,
                tc.tile_pool(name="ps3", bufs=6, space="PSUM") as ps3,
            ):
                for mt in range(NKC):
                    for n2 in range(2):
                        pc = ps3.tile([128, QC], f32, tag="pc")
                        for kd in range(4):
                            nc.tensor.matmul(
                                out=pc[:],
                                lhsT=ot[:, kd, mt * 128:(mt + 1) * 128],
                                rhs=wc_sb[:, kd, n2 * QC:(n2 + 1) * QC],
                                start=(kd == 0), stop=(kd == 3))
                        st = stp.tile([128, QC], bf16, tag="st")
                        if (mt + n2) % 2 == 0:
                            nc.vector.tensor_copy(st[:], pc[:])
                        else:
                            nc.scalar.copy(st[:], pc[:])
                        # out is [2, T, QC] (host reassembles) so each store
                        # writes one fully contiguous 128 KB block
                        nc.sync.dma_start(
                            out=out[n2, mt * 128:(mt + 1) * 128, :],
                            in_=st[:])


def build_program():
    """Build and compile the per-core Bass program (cached)."""
    if "nc" in _CACHE:
        return _CACHE["nc"]
    import concourse.bacc as bacc
    import concourse.tile as tile
    from concourse import mybir

    f32 = mybir.dt.float32
    bf16 = mybir.dt.bfloat16
    nc = bacc.Bacc("TRN2", target_bir_lowering=False, debug=False,
                   num_devices=N_CORES)
    io = {
        "xT": nc.dram_tensor("xT", [NQC, C, QC], bf16,
                             kind="ExternalInput").ap(),
        "wq": nc.dram_tensor("wq", [C, HL], bf16, kind="ExternalInput").ap(),
        "wk": nc.dram_tensor("wk", [C, HL], bf16, kind="ExternalInput").ap(),
        "wv": nc.dram_tensor("wv", [C, HL], bf16, kind="ExternalInput").ap(),
        "wc": nc.dram_tensor("wc", [HL, C], bf16, kind="ExternalInput").ap(),
        "maskw": nc.dram_tensor("maskw", [128, 2, 128], bf16,
                                kind="ExternalInput").ap(),
        "out": nc.dram_tensor("out", [2, T, QC], bf16,
                              kind="ExternalOutput").ap(),
    }
    with tile.TileContext(nc) as tc:
        _emit(nc, tc, tile, mybir, io)
    nc.compile()
    _CACHE["nc"] = nc
    return nc


def make_in_maps(x, Wq, Wk, Wv, Wc):
    import ml_dtypes
    bf16 = ml_dtypes.bfloat16
    x = np.asarray(x, dtype=np.float32)
    Wq = np.asarray(Wq, dtype=np.float32).astype(bf16)
    Wk = np.asarray(Wk, dtype=np.float32).astype(bf16)
    Wv = np.asarray(Wv, dtype=np.float32).astype(bf16)
    Wc = np.asarray(Wc, dtype=np.float32).astype(bf16)

    # causal triangle for the leading 128-wide diagonal block (after
    # truncating fully-masked columns), duplicated for the head pair
    i_idx = np.arange(128)[:, None]
    j_idx = np.arange(128)[None, :]
    tri = (j_idx >= i_idx).astype(bf16)          # [128, 128]
    maskw = np.repeat(tri[:, None, :], 2, axis=1)  # [128, 2, 128]

    in_maps = []
    for b in range(B):
        # pre-chunk x^T to [NQC, C, QC] so each on-device x read is a
        # single contiguous block (x^T row windows are strided otherwise)
        xT = np.ascontiguousarray(
            x[b].T.reshape(C, NQC, QC).transpose(1, 0, 2)).astype(bf16)
        for g in range(2):
            sl = slice(g * HL, (g + 1) * HL)
            in_maps.append({
                "xT": xT,
                "wq": np.ascontiguousarray(Wq[:, sl]),
                "wk": np.ascontiguousarray(Wk[:, sl]),
                "wv": np.ascontiguousarray(Wv[:, sl]),
                "wc": np.ascontiguousarray(Wc[sl, :]),
                "maskw": maskw,
            })
    return in_maps


def kernel(x, Wq, Wk, Wv, Wc, bc):
    from concourse.bass_utils import run_bass_kernel_spmd

    nc = build_program()
    in_maps = make_in_maps(x, Wq, Wk, Wv, Wc)
    res = run_bass_kernel_spmd(nc, in_maps, core_ids=list(range(N_CORES)))
    bc = np.asarray(bc, dtype=np.float32)
    out = np.empty((B, T, C), dtype=np.float32)
    for b in range(B):
        # device out is [2, T, QC] (two contiguous column halves)
        o0 = res.results[2 * b]["out"].astype(np.float32)
        o1 = res.results[2 * b + 1]["out"].astype(np.float32)
        o = o0 + o1
        out[b] = np.concatenate([o[0], o[1]], axis=1) + bc
    return out



# revision 60
# speedup vs baseline: 1.0431x; 1.0431x over previous
"""Trainium2 Bass kernel for 16-head causal MultiHeadAttention.

Problem shapes (hardcoded): x [4, 2048, 1024], Wq/Wk/Wv [1024, 1024],
Wc [1024, 1024], bc [1024].  Output [4, 2048, 1024].

Sharding: 8 cores = (batch b in 0..3) x (head-group g in 0..1).
Each core computes 8 heads (512 of the 1024 hidden dims) for one batch
element, including its partial c_proj contribution.  The host sums the
two partials per batch (fp32) and adds the bias.

All matmul operands are bf16 (host-cast inputs; fp32 PSUM accumulate;
bf16 output summed in fp32 on the host) — rel err ~6e-3 vs the 2e-2
gate, and bf16 halves LDWEIGHTS/SBUF/DMA traffic and power-throttling
pressure vs fp32r.

Per-core kernel:
  P1:  one pass over x^T computing Q^T, K^T = W^T @ x_b^T [512, 2048]
       and V = x_b @ Wv_g (stored with a ones column per head).
       Startup: 2-kc-wide x/weight tiles, weights spread over the
       scalar/gpsimd/sync DMA rings interleaved with x so the first
       matmul only waits for its own slices.
  P2:  per (head-pair, q-chunk, k-chunk): the two heads' 64-row S^T
       matmuls are adjacent and PE-array-tile paired (rows 0-63/64-127
       concurrently) -> joint exp over both heads (scale 1/8) -> causal
       triangle mask (diagonal blocks only) -> O^T/denominator
       accumulate via [V | 1] stationary.  Diagonal k-chunks truncate
       the fully-masked leading q-columns, shrinking S/exp/O by ~15%.
       Eviction reads PSUM once (fast bank recycling), then reciprocal
       on the 1-row denominator + 1 KB DRAM-bounce broadcast of the
       bf16 reciprocal, all off the PSUM critical path.
  P3:  partial out = O @ Wc_g   (O^T chunks are the matmul stationaries)
"""

import numpy as np

B, T, C = 4, 2048, 1024
H_PER_CORE = 8       # heads per core
HL = 512             # local head width  (8 heads * 64)
D = 64               # head dim
QC = 512             # q-chunk width (moving free dim)
NQC = T // QC        # 4
NKC = T // 128       # 16
N_CORES = 8

_CACHE = {}


def _emit(nc, tc, tile, mybir, io):
    import concourse.bass as bass
    f32, bf16 = mybir.dt.float32, mybir.dt.bfloat16
    Exp = mybir.ActivationFunctionType.Exp
    xT, wq, wk, wv, wc, maskw, out = (
        io["xT"], io["wq"], io["wk"], io["wv"], io["wc"],
        io["maskw"], io["out"],
    )

    from contextlib import ExitStack

    with ExitStack() as ctx:
        persist = ctx.enter_context(tc.tile_pool(name="persist", bufs=1))
        # Q^T / K^T / O^T: [512 rows, 2048 toks] as [128, 4 chunks, 2048]
        qt = persist.tile([128, 4, T], bf16)
        kt = persist.tile([128, 4, T], bf16)
        # V': [2048 toks, 8 heads x (64 dims + ones col)] as [128, 16, 520]
        vp = persist.tile([128, NKC, H_PER_CORE * (D + 1)], bf16)
        vp4 = vp.rearrange("p mt (h c) -> p mt h c", c=D + 1)
        # ones column per head (denominator accumulator in the O matmuls)
        nc.gpsimd.memset(vp4[:, :, :, D], 1.0)

        # ------- Phase 1: Q^T, K^T, V in one pass over x^T -------
        with (
            tc.tile_pool(name="wqk", bufs=1) as wpool,
            tc.tile_pool(name="xtp", bufs=8) as xtp,
            tc.tile_pool(name="ps1", bufs=6, space="PSUM") as ps1,
        ):
            wq_t = [wpool.tile([128, 2, HL], bf16, tag=f"wq{i}", name=f"wq{i}")
                    for i in range(4)]
            wk_t = [wpool.tile([128, 2, HL], bf16, tag=f"wk{i}", name=f"wk{i}")
                    for i in range(4)]
            wv_t = [wpool.tile([128, 2, HL], bf16, tag=f"wv{i}", name=f"wv{i}")
                    for i in range(4)]

            def load_xt(n, with_weights=False):
                """x^T [1024, 512-tok chunk n] as 4 two-kc tiles on the sync
                DMA ring; on the first chunk the weight loads ride the
                scalar/vector/tensor rings in parallel, interleaved so
                matmul kc only waits for its own slice of x and W."""
                xts = []
                for i in range(4):
                    t = xtp.tile([128, 2, QC], bf16, tag="xt")
                    # xT is pre-chunked host-side to [NQC, C, QC] so this
                    # reads one fully contiguous 256 KB block
                    nc.sync.dma_start(
                        out=t,
                        in_=xT[n, i * 256:(i + 1) * 256, :]
                        .rearrange("(c p) m -> p c m", p=128))
                    if with_weights:
                        sl = slice(i * 256, (i + 1) * 256)
                        nc.scalar.dma_start(
                            out=wq_t[i],
                            in_=wq[sl, :].rearrange("(c p) m -> p c m", p=128))
                        nc.gpsimd.dma_start(
                            out=wk_t[i],
                            in_=wk[sl, :].rearrange("(c p) m -> p c m", p=128))
                    xts.append(t)
                if with_weights:
                    # wv rides the sync ring after x: needed last (V matmuls
                    # follow Q/K per n) — keeps all three rings ~1 MB each
                    for i in range(4):
                        sl = slice(i * 256, (i + 1) * 256)
                        nc.sync.dma_start(
                            out=wv_t[i],
                            in_=wv[sl, :].rearrange("(c p) m -> p c m", p=128))
                return lambda kc: xts[kc // 2][:, kc % 2, :]

            def wsl(wt, kc):
                return wt[kc // 2][:, kc % 2, :]

            for n in range(NQC):
                xs = load_xt(n, with_weights=(n == 0))
                # all Q tiles first: ~13us of PE work gated only on wq+x,
                # hiding the slower wk (SWDGE) / wv transfer latency
                for mc in range(4):
                    pq = ps1.tile([128, QC], f32, tag="p1")
                    for kc in range(8):
                        nc.tensor.matmul(
                            out=pq[:],
                            lhsT=wsl(wq_t, kc)[:, mc * 128:(mc + 1) * 128],
                            rhs=xs(kc), start=(kc == 0), stop=(kc == 7))
                    nc.scalar.copy(qt[:, mc, n * QC:(n + 1) * QC], pq[:])
                for mc in range(4):
                    pk = ps1.tile([128, QC], f32, tag="p1")
                    for kc in range(8):
                        nc.tensor.matmul(
                            out=pk[:],
                            lhsT=wsl(wk_t, kc)[:, mc * 128:(mc + 1) * 128],
                            rhs=xs(kc), start=(kc == 0), stop=(kc == 7))
                    nc.vector.tensor_copy(kt[:, mc, n * QC:(n + 1) * QC], pk[:])
                for mt in range(4):
                    gm = n * 4 + mt           # global token chunk (0..15)
                    pv = ps1.tile([128, HL], f32, tag="p1")
                    for kc in range(8):
                        nc.tensor.matmul(
                            out=pv[:], lhsT=xs(kc)[:, mt * 128:(mt + 1) * 128],
                            rhs=wsl(wv_t, kc), start=(kc == 0), stop=(kc == 7))
                    nc.vector.tensor_copy(
                        vp4[:, gm, :, 0:D],
                        pv.rearrange("p (h d) -> p h d", d=D))


        # ---------------- Phase 2: attention ----------------
        with tc.tile_pool(name="otp", bufs=1) as otpool, \
             tc.tile_pool(name="wcp", bufs=1) as wcpool:
            ot = otpool.tile([128, 4, T], bf16)
            # preload Wc during P2 (used in P3)
            wc_sb = wcpool.tile([128, 4, C], bf16)
            nc.sync.dma_start(
                out=wc_sb, in_=wc.rearrange("(kd p) m -> p kd m", p=128))

            with (
                tc.tile_pool(name="mk", bufs=1) as mkpool,
                tc.tile_pool(name="etp", bufs=8) as etp,
                tc.tile_pool(name="smp", bufs=6) as smp,
                tc.tile_pool(name="drp", bufs=8, space="DRAM") as drp,
                tc.tile_pool(name="psw", bufs=3, space="PSUM") as psw,
                tc.tile_pool(name="pso", bufs=2, space="PSUM") as pso,
            ):
                # causal triangle for the 128-wide diagonal block,
                # duplicated so one tensor_mul masks a head pair at once
                mask_sb = mkpool.tile([128, 2, 128], bf16)
                nc.sync.dma_start(out=mask_sb, in_=maskw)

                def emit_chunk(ha, hb, qc, po_a, po_b, kc, K):
                    """One k-chunk for a head pair: adjacent 64-row S-mms
                    (PE-tile paired), joint exp + mask, two O-mms.

                    Diagonal chunks (d >= 0) skip the fully-masked leading
                    q-columns: only q >= 128*d can attend to this chunk, so
                    the S/exp/O work all shrink; just the leading 128-wide
                    block needs the causal triangle mask."""
                    d = kc - 4 * qc
                    off = max(d, 0) * 128
                    N = QC - off
                    pw = psw.tile([128, 2, QC], f32, tag="pw")
                    for j, h in ((0, ha), (1, hb)):
                        r0 = (h % 2) * 64
                        chh = h // 2
                        # 64-row array tiling: even heads use PE rows 0-63,
                        # odd heads rows 64-127 — the two adjacent S-matmuls
                        # run concurrently on the two halves.
                        nc.tensor.matmul(
                            out=pw[:, j, 0:N],
                            lhsT=kt[r0:r0 + 64, chh, kc * 128:(kc + 1) * 128],
                            rhs=qt[r0:r0 + 64, chh,
                                   qc * QC + off:(qc + 1) * QC],
                            start=True, stop=True, tile_position=(r0, 0))
                    ew = etp.tile([128, 2, QC], bf16, tag="et")
                    nc.scalar.activation(ew[:, :, 0:N], pw[:, :, 0:N],
                                         Exp, scale=0.125)
                    if d >= 0:               # diagonal block: causal triangle
                        nc.vector.tensor_mul(
                            ew[:, :, 0:128], ew[:, :, 0:128], mask_sb[:])
                    for j, h, po in ((0, ha, po_a), (1, hb, po_b)):
                        nc.tensor.matmul(
                            out=po[0:D + 1, off:QC],
                            lhsT=vp[:, kc, h * (D + 1):(h + 1) * (D + 1)],
                            rhs=ew[:, j, 0:N],
                            start=(kc == 0), stop=(kc == K - 1))

                onesrow = mkpool.tile([1, 64], bf16)
                nc.vector.memset(onesrow[:], 1.0)

                def evict(h, qc, po, fast=False):
                    """PSUM eviction with a single po read (frees the PSUM
                    bank after one op) + off-critical-path normalization.

                    Reciprocal runs on the single denominator row, then the
                    bf16 reciprocal is broadcast via a tiny DRAM bounce
                    (1 KB) instead of broadcasting the raw denominator and
                    computing 128 reciprocal rows.  The last evicts gate the
                    final c_proj tiles, so `fast` broadcasts via a small PE
                    matmul (the PE is idle there) instead of two DMA hops."""
                    r0 = (h % 2) * 64
                    chh = h // 2
                    ot_slice = ot[r0:r0 + 64, chh, qc * QC:(qc + 1) * QC]
                    og = smp.tile([65, QC], f32, tag="og")
                    nc.vector.tensor_copy(og[:], po[0:D + 1, :])
                    d1 = smp.tile([1, QC], f32, tag="d1")
                    nc.vector.tensor_copy(d1[:], og[D:D + 1, :])
                    nc.vector.reciprocal_approx_fast(d1[:], d1[:])
                    dr = smp.tile([1, QC], bf16, tag="dr")
                    nc.vector.tensor_copy(dr[:], d1[:])
                    if fast:
                        pb = psw.tile([128, 2, QC], f32, tag="pw")
                        nc.tensor.matmul(
                            out=pb[0:64, 0, :], lhsT=onesrow[:],
                            rhs=dr[:], start=True, stop=True)
                        nc.vector.tensor_mul(ot_slice, og[0:64, :],
                                             pb[0:64, 0, :])
                        return
                    scr = drp.tile([1, QC], bf16, tag="scr")
                    nc.sync.dma_start(out=scr[:], in_=dr[:])
                    db = smp.tile([64, QC], bf16, tag="db")
                    s0 = scr[:]
                    nc.gpsimd.dma_start(
                        out=db[:],
                        in_=bass.AP(tensor=s0.tensor, offset=s0.offset,
                                    ap=[[0, 64], [1, QC]]))
                    # normalize on gpsimd: keeps the DMA-gated multiply off
                    # the vector queue so the next pair's PSUM evictions
                    # (vector) aren't stuck behind the DRAM-bounce latency
                    nc.gpsimd.tensor_mul(ot_slice, og[0:64, :], db[:])

                # Head pairs in chunk-lockstep so the two heads' 64-row
                # S-matmuls are adjacent and fill both PE array halves.
                for hp in range(H_PER_CORE // 2):
                    ha, hb = 2 * hp, 2 * hp + 1
                    for qc in range(NQC):
                        K = 4 * qc + 4      # causal k-chunks for this q-chunk
                        po_a = pso.tile([128, QC], f32, tag="po")
                        po_b = pso.tile([128, QC], f32, tag="po")
                        for kc in range(K):
                            emit_chunk(ha, hb, qc, po_a, po_b, kc, K)
                        evict(ha, qc, po_a)
                        evict(hb, qc, po_b)

            # ---------------- Phase 3: c_proj partial ----------------
            with (
                tc.tile_pool(name="stp", bufs=4) as stp,
                tc.tile_pool(name="ps3", bufs=6, space="PSUM") as ps3,
            ):
                for mt in range(NKC):
                    for n2 in range(2):
                        pc = ps3.tile([128, QC], f32, tag="pc")
                        for kd in range(4):
                            nc.tensor.matmul(
                                out=pc[:],
                                lhsT=ot[:, kd, mt * 128:(mt + 1) * 128],
                                rhs=wc_sb[:, kd, n2 * QC:(n2 + 1) * QC],
                                start=(kd == 0), stop=(kd == 3))
                        st = stp.tile([128, QC], bf16, tag="st")
                        # out is [2, T, QC] (host reassembles) so each store
                        # writes one fully contiguous 128 KB block; copies
                        # and stores alternate engine rings to halve the
                        # serialized issue cost at the drain
                        if (mt + n2) % 2 == 0:
                            nc.vector.tensor_copy(st[:], pc[:])
                            nc.sync.dma_start(
                                out=out[n2, mt * 128:(mt + 1) * 128, :],
                                in_=st[:])
                        else:
                            nc.scalar.copy(st[:], pc[:])
                            nc.scalar.dma_start(
                                out=out[n2, mt * 128:(mt + 1) * 128, :],
                                in_=st[:])


def build_program():
    """Build and compile the per-core Bass program (cached)."""
    if "nc" in _CACHE:
        return _CACHE["nc"]
    import concourse.bacc as bacc
    import concourse.tile as tile
    from concourse import mybir

    f32 = mybir.dt.float32
    bf16 = mybir.dt.bfloat16
    nc = bacc.Bacc("TRN2", target_bir_lowering=False, debug=False,
                   num_devices=N_CORES)
    io = {
        "xT": nc.dram_tensor("xT", [NQC, C, QC], bf16,
                             kind="ExternalInput").ap(),
        "wq": nc.dram_tensor("wq", [C, HL], bf16, kind="ExternalInput").ap(),
        "wk": nc.dram_tensor("wk", [C, HL], bf16, kind="ExternalInput").ap(),
        "wv": nc.dram_tensor("wv", [C, HL], bf16, kind="ExternalInput").ap(),
        "wc": nc.dram_tensor("wc", [HL, C], bf16, kind="ExternalInput").ap(),
        "maskw": nc.dram_tensor("maskw", [128, 2, 128], bf16,
                                kind="ExternalInput").ap(),
        "out": nc.dram_tensor("out", [2, T, QC], bf16,
                              kind="ExternalOutput").ap(),
    }
    with tile.TileContext(nc) as tc:
        _emit(nc, tc, tile, mybir, io)
    nc.compile()
    _CACHE["nc"] = nc
    return nc


def make_in_maps(x, Wq, Wk, Wv, Wc):
    import ml_dtypes
    bf16 = ml_dtypes.bfloat16
    x = np.asarray(x, dtype=np.float32)
    Wq = np.asarray(Wq, dtype=np.float32).astype(bf16)
    Wk = np.asarray(Wk, dtype=np.float32).astype(bf16)
    Wv = np.asarray(Wv, dtype=np.float32).astype(bf16)
    Wc = np.asarray(Wc, dtype=np.float32).astype(bf16)

    # causal triangle for the leading 128-wide diagonal block (after
    # truncating fully-masked columns), duplicated for the head pair
    i_idx = np.arange(128)[:, None]
    j_idx = np.arange(128)[None, :]
    tri = (j_idx >= i_idx).astype(bf16)          # [128, 128]
    maskw = np.repeat(tri[:, None, :], 2, axis=1)  # [128, 2, 128]

    in_maps = []
    for b in range(B):
        # pre-chunk x^T to [NQC, C, QC] so each on-device x read is a
        # single contiguous block (x^T row windows are strided otherwise)
        xT = np.ascontiguousarray(
            x[b].T.reshape(C, NQC, QC).transpose(1, 0, 2)).astype(bf16)
        for g in range(2):
            sl = slice(g * HL, (g + 1) * HL)
            in_maps.append({
                "xT": xT,
                "wq": np.ascontiguousarray(Wq[:, sl]),
                "wk": np.ascontiguousarray(Wk[:, sl]),
                "wv": np.ascontiguousarray(Wv[:, sl]),
                "wc": np.ascontiguousarray(Wc[sl, :]),
                "maskw": maskw,
            })
    return in_maps


def kernel(x, Wq, Wk, Wv, Wc, bc):
    from concourse.bass_utils import run_bass_kernel_spmd

    nc = build_program()
    in_maps = make_in_maps(x, Wq, Wk, Wv, Wc)
    res = run_bass_kernel_spmd(nc, in_maps, core_ids=list(range(N_CORES)))
    bc = np.asarray(bc, dtype=np.float32)
    out = np.empty((B, T, C), dtype=np.float32)
    for b in range(B):
        # device out is [2, T, QC] (two contiguous column halves)
        o0 = res.results[2 * b]["out"].astype(np.float32)
        o1 = res.results[2 * b + 1]["out"].astype(np.float32)
        o = o0 + o1
        out[b] = np.concatenate([o[0], o[1]], axis=1) + bc
    return out



# revision 61
# speedup vs baseline: 1.0455x; 1.0023x over previous
"""Trainium2 Bass kernel for 16-head causal MultiHeadAttention.

Problem shapes (hardcoded): x [4, 2048, 1024], Wq/Wk/Wv [1024, 1024],
Wc [1024, 1024], bc [1024].  Output [4, 2048, 1024].

Sharding: 8 cores = (batch b in 0..3) x (head-group g in 0..1).
Each core computes 8 heads (512 of the 1024 hidden dims) for one batch
element, including its partial c_proj contribution.  The host sums the
two partials per batch (fp32) and adds the bias.

All matmul operands are bf16 (host-cast inputs; fp32 PSUM accumulate;
bf16 output summed in fp32 on the host) — rel err ~6e-3 vs the 2e-2
gate, and bf16 halves LDWEIGHTS/SBUF/DMA traffic and power-throttling
pressure vs fp32r.

Per-core kernel:
  P1:  one pass over x^T computing Q^T, K^T = W^T @ x_b^T [512, 2048]
       and V = x_b @ Wv_g (stored with a ones column per head).
       Startup: 2-kc-wide x/weight tiles, weights spread over the
       scalar/gpsimd/sync DMA rings interleaved with x so the first
       matmul only waits for its own slices.
  P2:  per (head-pair, q-chunk, k-chunk): the two heads' 64-row S^T
       matmuls are adjacent and PE-array-tile paired (rows 0-63/64-127
       concurrently) -> joint exp over both heads (scale 1/8) -> causal
       triangle mask (diagonal blocks only) -> O^T/denominator
       accumulate via [V | 1] stationary.  Diagonal k-chunks truncate
       the fully-masked leading q-columns, shrinking S/exp/O by ~15%.
       Eviction reads PSUM once (fast bank recycling), then reciprocal
       on the 1-row denominator + 1 KB DRAM-bounce broadcast of the
       bf16 reciprocal, all off the PSUM critical path.
  P3:  partial out = O @ Wc_g   (O^T chunks are the matmul stationaries)
"""

import numpy as np

B, T, C = 4, 2048, 1024
H_PER_CORE = 8       # heads per core
HL = 512             # local head width  (8 heads * 64)
D = 64               # head dim
QC = 512             # q-chunk width (moving free dim)
NQC = T // QC        # 4
NKC = T // 128       # 16
N_CORES = 8

_CACHE = {}


def _emit(nc, tc, tile, mybir, io):
    import concourse.bass as bass
    f32, bf16 = mybir.dt.float32, mybir.dt.bfloat16
    Exp = mybir.ActivationFunctionType.Exp
    xT, wq, wk, wv, wc, maskw, out = (
        io["xT"], io["wq"], io["wk"], io["wv"], io["wc"],
        io["maskw"], io["out"],
    )

    from contextlib import ExitStack

    with ExitStack() as ctx:
        persist = ctx.enter_context(tc.tile_pool(name="persist", bufs=1))
        # Q^T / K^T / O^T: [512 rows, 2048 toks] as [128, 4 chunks, 2048]
        qt = persist.tile([128, 4, T], bf16)
        kt = persist.tile([128, 4, T], bf16)
        # V': [2048 toks, 8 heads x (64 dims + ones col)] as [128, 16, 520]
        vp = persist.tile([128, NKC, H_PER_CORE * (D + 1)], bf16)
        vp4 = vp.rearrange("p mt (h c) -> p mt h c", c=D + 1)
        # ones column per head (denominator accumulator in the O matmuls)
        nc.gpsimd.memset(vp4[:, :, :, D], 1.0)

        # ------- Phase 1: Q^T, K^T, V in one pass over x^T -------
        with (
            tc.tile_pool(name="wqk", bufs=1) as wpool,
            tc.tile_pool(name="xtp", bufs=8) as xtp,
            tc.tile_pool(name="ps1", bufs=6, space="PSUM") as ps1,
        ):
            wq_t = [wpool.tile([128, 2, HL], bf16, tag=f"wq{i}", name=f"wq{i}")
                    for i in range(4)]
            wk_t = [wpool.tile([128, 2, HL], bf16, tag=f"wk{i}", name=f"wk{i}")
                    for i in range(4)]
            wv_t = [wpool.tile([128, 2, HL], bf16, tag=f"wv{i}", name=f"wv{i}")
                    for i in range(4)]

            def load_xt(n, with_weights=False):
                """x^T [1024, 512-tok chunk n] as 4 two-kc contiguous tiles
                (xT is pre-chunked host-side to [NQC, C, QC]).

                First chunk: wq rides the scalar ring first (Q matmuls come
                first), x-even on sync, x-odd on scalar behind wq, wk on
                gpsimd, wv on sync behind x — three rings in parallel so
                every P1 input for n=0 lands by ~13us instead of draining
                one ring serially."""
                xts = [None] * 4
                if not with_weights:
                    for i in range(4):
                        t = xtp.tile([128, 2, QC], bf16, tag="xt")
                        nc.sync.dma_start(
                            out=t,
                            in_=xT[n, i * 256:(i + 1) * 256, :]
                            .rearrange("(c p) m -> p c m", p=128))
                        xts[i] = t
                    return lambda kc: xts[kc // 2][:, kc % 2, :]
                for i in range(4):
                    sl = slice(i * 256, (i + 1) * 256)
                    nc.scalar.dma_start(
                        out=wq_t[i],
                        in_=wq[sl, :].rearrange("(c p) m -> p c m", p=128))
                    nc.gpsimd.dma_start(
                        out=wk_t[i],
                        in_=wk[sl, :].rearrange("(c p) m -> p c m", p=128))
                for i in range(4):
                    t = xtp.tile([128, 2, QC], bf16, tag="xt")
                    ring = nc.sync if i % 2 == 0 else nc.scalar
                    ring.dma_start(
                        out=t,
                        in_=xT[n, i * 256:(i + 1) * 256, :]
                        .rearrange("(c p) m -> p c m", p=128))
                    xts[i] = t
                for i in range(4):
                    sl = slice(i * 256, (i + 1) * 256)
                    nc.sync.dma_start(
                        out=wv_t[i],
                        in_=wv[sl, :].rearrange("(c p) m -> p c m", p=128))
                return lambda kc: xts[kc // 2][:, kc % 2, :]

            def wsl(wt, kc):
                return wt[kc // 2][:, kc % 2, :]

            for n in range(NQC):
                xs = load_xt(n, with_weights=(n == 0))
                # all Q tiles first: ~13us of PE work gated only on wq+x,
                # hiding the slower wk (SWDGE) / wv transfer latency
                for mc in range(4):
                    pq = ps1.tile([128, QC], f32, tag="p1")
                    for kc in range(8):
                        nc.tensor.matmul(
                            out=pq[:],
                            lhsT=wsl(wq_t, kc)[:, mc * 128:(mc + 1) * 128],
                            rhs=xs(kc), start=(kc == 0), stop=(kc == 7))
                    nc.scalar.copy(qt[:, mc, n * QC:(n + 1) * QC], pq[:])
                for mc in range(4):
                    pk = ps1.tile([128, QC], f32, tag="p1")
                    for kc in range(8):
                        nc.tensor.matmul(
                            out=pk[:],
                            lhsT=wsl(wk_t, kc)[:, mc * 128:(mc + 1) * 128],
                            rhs=xs(kc), start=(kc == 0), stop=(kc == 7))
                    nc.vector.tensor_copy(kt[:, mc, n * QC:(n + 1) * QC], pk[:])
                for mt in range(4):
                    gm = n * 4 + mt           # global token chunk (0..15)
                    pv = ps1.tile([128, HL], f32, tag="p1")
                    for kc in range(8):
                        nc.tensor.matmul(
                            out=pv[:], lhsT=xs(kc)[:, mt * 128:(mt + 1) * 128],
                            rhs=wsl(wv_t, kc), start=(kc == 0), stop=(kc == 7))
                    nc.vector.tensor_copy(
                        vp4[:, gm, :, 0:D],
                        pv.rearrange("p (h d) -> p h d", d=D))


        # ---------------- Phase 2: attention ----------------
        with tc.tile_pool(name="otp", bufs=1) as otpool, \
             tc.tile_pool(name="wcp", bufs=1) as wcpool:
            ot = otpool.tile([128, 4, T], bf16)
            # preload Wc during P2 (used in P3)
            wc_sb = wcpool.tile([128, 4, C], bf16)
            nc.sync.dma_start(
                out=wc_sb, in_=wc.rearrange("(kd p) m -> p kd m", p=128))

            with (
                tc.tile_pool(name="mk", bufs=1) as mkpool,
                tc.tile_pool(name="etp", bufs=8) as etp,
                tc.tile_pool(name="smp", bufs=6) as smp,
                tc.tile_pool(name="drp", bufs=8, space="DRAM") as drp,
                tc.tile_pool(name="psw", bufs=3, space="PSUM") as psw,
                tc.tile_pool(name="pso", bufs=2, space="PSUM") as pso,
            ):
                # causal triangle for the 128-wide diagonal block,
                # duplicated so one tensor_mul masks a head pair at once
                mask_sb = mkpool.tile([128, 2, 128], bf16)
                nc.sync.dma_start(out=mask_sb, in_=maskw)

                def emit_chunk(ha, hb, qc, po_a, po_b, kc, K):
                    """One k-chunk for a head pair: adjacent 64-row S-mms
                    (PE-tile paired), joint exp + mask, two O-mms.

                    Diagonal chunks (d >= 0) skip the fully-masked leading
                    q-columns: only q >= 128*d can attend to this chunk, so
                    the S/exp/O work all shrink; just the leading 128-wide
                    block needs the causal triangle mask."""
                    d = kc - 4 * qc
                    off = max(d, 0) * 128
                    N = QC - off
                    pw = psw.tile([128, 2, QC], f32, tag="pw")
                    for j, h in ((0, ha), (1, hb)):
                        r0 = (h % 2) * 64
                        chh = h // 2
                        # 64-row array tiling: even heads use PE rows 0-63,
                        # odd heads rows 64-127 — the two adjacent S-matmuls
                        # run concurrently on the two halves.
                        nc.tensor.matmul(
                            out=pw[:, j, 0:N],
                            lhsT=kt[r0:r0 + 64, chh, kc * 128:(kc + 1) * 128],
                            rhs=qt[r0:r0 + 64, chh,
                                   qc * QC + off:(qc + 1) * QC],
                            start=True, stop=True, tile_position=(r0, 0))
                    ew = etp.tile([128, 2, QC], bf16, tag="et")
                    nc.scalar.activation(ew[:, :, 0:N], pw[:, :, 0:N],
                                         Exp, scale=0.125)
                    if d >= 0:               # diagonal block: causal triangle
                        nc.vector.tensor_mul(
                            ew[:, :, 0:128], ew[:, :, 0:128], mask_sb[:])
                    for j, h, po in ((0, ha, po_a), (1, hb, po_b)):
                        nc.tensor.matmul(
                            out=po[0:D + 1, off:QC],
                            lhsT=vp[:, kc, h * (D + 1):(h + 1) * (D + 1)],
                            rhs=ew[:, j, 0:N],
                            start=(kc == 0), stop=(kc == K - 1))

                onesrow = mkpool.tile([1, 64], bf16)
                nc.vector.memset(onesrow[:], 1.0)

                def evict(h, qc, po, fast=False):
                    """PSUM eviction with a single po read (frees the PSUM
                    bank after one op) + off-critical-path normalization.

                    Reciprocal runs on the single denominator row, then the
                    bf16 reciprocal is broadcast via a tiny DRAM bounce
                    (1 KB) instead of broadcasting the raw denominator and
                    computing 128 reciprocal rows.  The last evicts gate the
                    final c_proj tiles, so `fast` broadcasts via a small PE
                    matmul (the PE is idle there) instead of two DMA hops."""
                    r0 = (h % 2) * 64
                    chh = h // 2
                    ot_slice = ot[r0:r0 + 64, chh, qc * QC:(qc + 1) * QC]
                    og = smp.tile([65, QC], f32, tag="og")
                    nc.vector.tensor_copy(og[:], po[0:D + 1, :])
                    d1 = smp.tile([1, QC], f32, tag="d1")
                    nc.vector.tensor_copy(d1[:], og[D:D + 1, :])
                    nc.vector.reciprocal_approx_fast(d1[:], d1[:])
                    dr = smp.tile([1, QC], bf16, tag="dr")
                    nc.vector.tensor_copy(dr[:], d1[:])
                    if fast:
                        pb = psw.tile([128, 2, QC], f32, tag="pw")
                        nc.tensor.matmul(
                            out=pb[0:64, 0, :], lhsT=onesrow[:],
                            rhs=dr[:], start=True, stop=True)
                        nc.vector.tensor_mul(ot_slice, og[0:64, :],
                                             pb[0:64, 0, :])
                        return
                    scr = drp.tile([1, QC], bf16, tag="scr")
                    nc.sync.dma_start(out=scr[:], in_=dr[:])
                    db = smp.tile([64, QC], bf16, tag="db")
                    s0 = scr[:]
                    nc.gpsimd.dma_start(
                        out=db[:],
                        in_=bass.AP(tensor=s0.tensor, offset=s0.offset,
                                    ap=[[0, 64], [1, QC]]))
                    # normalize on gpsimd: keeps the DMA-gated multiply off
                    # the vector queue so the next pair's PSUM evictions
                    # (vector) aren't stuck behind the DRAM-bounce latency
                    nc.gpsimd.tensor_mul(ot_slice, og[0:64, :], db[:])

                # Head pairs in chunk-lockstep so the two heads' 64-row
                # S-matmuls are adjacent and fill both PE array halves.
                for hp in range(H_PER_CORE // 2):
                    ha, hb = 2 * hp, 2 * hp + 1
                    for qc in range(NQC):
                        K = 4 * qc + 4      # causal k-chunks for this q-chunk
                        po_a = pso.tile([128, QC], f32, tag="po")
                        po_b = pso.tile([128, QC], f32, tag="po")
                        for kc in range(K):
                            emit_chunk(ha, hb, qc, po_a, po_b, kc, K)
                        evict(ha, qc, po_a)
                        evict(hb, qc, po_b)

            # ---------------- Phase 3: c_proj partial ----------------
            with (
                tc.tile_pool(name="stp", bufs=4) as stp,
                tc.tile_pool(name="ps3", bufs=6, space="PSUM") as ps3,
            ):
                for mt in range(NKC):
                    for n2 in range(2):
                        pc = ps3.tile([128, QC], f32, tag="pc")
                        for kd in range(4):
                            nc.tensor.matmul(
                                out=pc[:],
                                lhsT=ot[:, kd, mt * 128:(mt + 1) * 128],
                                rhs=wc_sb[:, kd, n2 * QC:(n2 + 1) * QC],
                                start=(kd == 0), stop=(kd == 3))
                        st = stp.tile([128, QC], bf16, tag="st")
                        # out is [2, T, QC] (host reassembles) so each store
                        # writes one fully contiguous 128 KB block; copies
                        # and stores alternate engine rings to halve the
                        # serialized issue cost at the drain
                        if (mt + n2) % 2 == 0:
                            nc.vector.tensor_copy(st[:], pc[:])
                            nc.sync.dma_start(
                                out=out[n2, mt * 128:(mt + 1) * 128, :],
                                in_=st[:])
                        else:
                            nc.scalar.copy(st[:], pc[:])
                            nc.scalar.dma_start(
                                out=out[n2, mt * 128:(mt + 1) * 128, :],
                                in_=st[:])


def build_program():
    """Build and compile the per-core Bass program (cached)."""
    if "nc" in _CACHE:
        return _CACHE["nc"]
    import concourse.bacc as bacc
    import concourse.tile as tile
    from concourse import mybir

    f32 = mybir.dt.float32
    bf16 = mybir.dt.bfloat16
    nc = bacc.Bacc("TRN2", target_bir_lowering=False, debug=False,
                   num_devices=N_CORES)
    io = {
        "xT": nc.dram_tensor("xT", [NQC, C, QC], bf16,
                             kind="ExternalInput").ap(),
        "wq": nc.dram_tensor("wq", [C, HL], bf16, kind="ExternalInput").ap(),
        "wk": nc.dram_tensor("wk", [C, HL], bf16, kind="ExternalInput").ap(),
        "wv": nc.dram_tensor("wv", [C, HL], bf16, kind="ExternalInput").ap(),
        "wc": nc.dram_tensor("wc", [HL, C], bf16, kind="ExternalInput").ap(),
        "maskw": nc.dram_tensor("maskw", [128, 2, 128], bf16,
                                kind="ExternalInput").ap(),
        "out": nc.dram_tensor("out", [2, T, QC], bf16,
                              kind="ExternalOutput").ap(),
    }
    with tile.TileContext(nc) as tc:
        _emit(nc, tc, tile, mybir, io)
    nc.compile()
    _CACHE["nc"] = nc
    return nc


def make_in_maps(x, Wq, Wk, Wv, Wc):
    import ml_dtypes
    bf16 = ml_dtypes.bfloat16
    x = np.asarray(x, dtype=np.float32)
    Wq = np.asarray(Wq, dtype=np.float32).astype(bf16)
    Wk = np.asarray(Wk, dtype=np.float32).astype(bf16)
    Wv = np.asarray(Wv, dtype=np.float32).astype(bf16)
    Wc = np.asarray(Wc, dtype=np.float32).astype(bf16)

    # causal triangle for the leading 128-wide diagonal block (after
    # truncating fully-masked columns), duplicated for the head pair
    i_idx = np.arange(128)[:, None]
    j_idx = np.arange(128)[None, :]
    tri = (j_idx >= i_idx).astype(bf16)          # [128, 128]
    maskw = np.repeat(tri[:, None, :], 2, axis=1)  # [128, 2, 128]

    in_maps = []
    for b in range(B):
        # pre-chunk x^T to [NQC, C, QC] so each on-device x read is a
        # single contiguous block (x^T row windows are strided otherwise)
        xT = np.ascontiguousarray(
            x[b].T.reshape(C, NQC, QC).transpose(1, 0, 2)).astype(bf16)
        for g in range(2):
            sl = slice(g * HL, (g + 1) * HL)
            in_maps.append({
                "xT": xT,
                "wq": np.ascontiguousarray(Wq[:, sl]),
                "wk": np.ascontiguousarray(Wk[:, sl]),
                "wv": np.ascontiguousarray(Wv[:, sl]),
                "wc": np.ascontiguousarray(Wc[sl, :]),
                "maskw": maskw,
            })
    return in_maps


def kernel(x, Wq, Wk, Wv, Wc, bc):
    from concourse.bass_utils import run_bass_kernel_spmd

    nc = build_program()
    in_maps = make_in_maps(x, Wq, Wk, Wv, Wc)
    res = run_bass_kernel_spmd(nc, in_maps, core_ids=list(range(N_CORES)))
    bc = np.asarray(bc, dtype=np.float32)
    out = np.empty((B, T, C), dtype=np.float32)
    for b in range(B):
        # device out is [2, T, QC] (two contiguous column halves)
        o0 = res.results[2 * b]["out"].astype(np.float32)
        o1 = res.results[2 * b + 1]["out"].astype(np.float32)
        o = o0 + o1
        out[b] = np.concatenate([o[0], o[1]], axis=1) + bc
    return out

